# revision 18
# baseline (speedup 1.0000x reference)
"""FCOS detection head on 8 Trainium2 NeuronCores (Bass/Tile), fp8 DoubleRow.

Data parallel: batch 16 -> 2 images per core. Weights replicated.

Per-core compute layout:
  - channels on SBUF partitions (256 ch -> DoubleRow-fused pair of 128-ch
    k-tiles), spatial flattened on the free dim, activations stored fp8(e4m3)
    zero-padded (H+2)x(W+2) in a 16B-aligned pitch so a 3x3 conv is 9
    DoubleRow matmuls (or 12 for the 1D F(2,3) Winograd form) per PSUM tile.
  - weights are scaled by 512 on the host before fp8 quantization; the
    epilogue activation applies 1/512.
  - stem layers run either 1D Winograd F(2,3) along W (input transform split
    DVE/GPSIMD, output transform DVE+GPSIMD+ACT) or direct 3x3 (no DVE work);
    the mix is chosen per tower/layer to balance PE vs DVE load.
  - layer-0 input transforms are precomputed on the HOST (they only depend on
    the network input) and DMA'd in, so layer 0 needs no on-chip transforms.
  - head outputs (85 = 80 cls + 4 box + 1 ctr channels) are assembled
    channels-on-partitions, then PE-transposed per 128-position chunk into
    (positions, 85) and DMA'd to HBM.
"""

import numpy as np
import ml_dtypes

import concourse.bacc as bacc
import concourse.bass as bass
import concourse.mybir as mybir
import concourse.tile as tile
from concourse.bass import ts
from concourse.bass_utils import run_bass_kernel_spmd
from concourse.masks import make_identity

F32 = mybir.dt.float32
F8 = mybir.dt.float8e4
NP8 = ml_dtypes.float8_e4m3
DRM = mybir.MatmulPerfMode.DoubleRow
WSCALE = 512.0
INV_WSCALE = float(1.0 / WSCALE)

N_CORES = 8
B_FULL = 16
BS = B_FULL // N_CORES  # images per core
C = 256
NCLS = 80
NBC = 16  # box+ctr head padded to 16 output channels (5 real)
SDEPTH = 4
TAPS = [(ky, kx) for ky in range(3) for kx in range(3)]

# Winograd/direct mode per (level, tower, layer). 'w' = F(2,3) wino, 'd' = direct.
# levels are emitted p5 first: its direct convs are pure-PE work that runs
# while the (larger) p3/p4 host-vt DMAs stream in, and it warms the PE clock.
# 'w' = 12-matmul F(2,3) wino; 'f' = 15-matmul wino variant that accumulates
# m1+(-m3) and m0+m1 in PSUM so the output transform is only 2 DVE ops;
# 'd' = direct 3x3.
WMODE = [
    ("dddd", "dddd"),  # p5
    ("wwww", "wfff"),  # p3 (cls, box)
    ("wwww", "wfff"),  # p4
]

LEVELS = [
    dict(H=16, W=16, R=16, Rw=None, base=5120, WPa=24, host_vt=False, feat="feat_p5"),
    dict(H=64, W=64, R=8, Rw=16, base=0, WPa=72, host_vt=True, feat="feat_p3"),
    dict(H=32, W=32, R=16, Rw=32, base=4096, WPa=40, host_vt=True, feat="feat_p4"),
]
HW_TOTAL = 64 * 64 + 32 * 32 + 16 * 16  # 5376


def build_nc():
    nc = bacc.Bacc()

    x_dram, vt_dram = {}, {}
    for i, lvl in enumerate(LEVELS):
        H, WPa = lvl["H"], lvl["WPa"]
        if lvl["host_vt"]:
            # host-precomputed F(2,3) input transform of x: (B, C, 4i, H+2, W/2)
            vt_dram[i] = nc.declare_dram_parameter(
                f"vtx_l{i}", [BS, C, 4, H + 2, (H and lvl["W"]) // 2], F8, isOutput=False
            )
        else:
            x_dram[i] = nc.declare_dram_parameter(
                f"x_l{i}", [BS, C, H + 2, WPa], F8, isOutput=False
            )
    # direct stem weights: (S, ci 128, ci_t 2, co_t*tap*co)
    w_cls_d = nc.declare_dram_parameter("w_cls_d", [SDEPTH, 128, 2, 2 * 9 * 128], F8, isOutput=False)
    w_box_d = nc.declare_dram_parameter("w_box_d", [SDEPTH, 128, 2, 2 * 9 * 128], F8, isOutput=False)
    # Winograd F(2,3)-transformed stem weights: (S, ci, ci_t, co_t*i5*dy3*co)
    # i-sections 0..3 are U0..U3; section 4 is -U3 (for PSUM-side accumulation)
    w_cls_w = nc.declare_dram_parameter("w_cls_w", [SDEPTH, 128, 2, 2 * 5 * 3 * 128], F8, isOutput=False)
    w_box_w = nc.declare_dram_parameter("w_box_w", [SDEPTH, 128, 2, 2 * 5 * 3 * 128], F8, isOutput=False)
    w_pcls = nc.declare_dram_parameter("w_pcls", [128, 2, 9 * NCLS], F8, isOutput=False)
    w_pbc = nc.declare_dram_parameter("w_pbc", [128, 2, 9 * NBC], F8, isOutput=False)
    b_stem = nc.declare_dram_parameter("b_stem", [128, 2 * SDEPTH * 2], F32, isOutput=False)
    b_pcls = nc.declare_dram_parameter("b_pcls", [NCLS, 1], F32, isOutput=False)
    b_pbc = nc.declare_dram_parameter("b_pbc", [NBC, 1], F32, isOutput=False)
    out_dram = nc.declare_dram_parameter("out", [BS, HW_TOTAL, 85], F32, isOutput=True)

    add = mybir.AluOpType.add
    sub = mybir.AluOpType.subtract
    mult = mybir.AluOpType.mult

    with tile.TileContext(nc) as tc:
        with (
            tc.tile_pool(name="const", bufs=1) as const,
            tc.tile_pool(name="wp", bufs=3) as wp,
            tc.tile_pool(name="acts", bufs=1) as acts,
            tc.tile_pool(name="stage", bufs=1) as stage,
            tc.tile_pool(name="pp", bufs=1, space="PSUM") as pp,
        ):
            ident = const.tile([128, 128], F32, name="ident")
            make_identity(nc, ident[:])
            bst = const.tile([128, 2, SDEPTH, 2, 1], F32, name="bst")
            nc.sync.dma_start(out=bst[:, :, :, :, 0], in_=b_stem[:].rearrange("p (t l c) -> p t l c", t=2, l=SDEPTH, c=2))
            bp_cls = const.tile([NCLS, 1], F32, name="bp_cls")
            nc.sync.dma_start(out=bp_cls[:], in_=b_pcls[:])
            bp_bc = const.tile([NBC, 1], F32, name="bp_bc")
            nc.sync.dma_start(out=bp_bc[:], in_=b_pbc[:])
            wpc = const.tile([128, 2, 9 * NCLS], F8, name="wpc")
            nc.sync.dma_start(out=wpc[:], in_=w_pcls[:])
            wpb = const.tile([128, 2, 9 * NBC], F8, name="wpb")
            nc.sync.dma_start(out=wpb[:], in_=w_pbc[:])

            for li, lvl in enumerate(LEVELS):
                H, W, R, Rw, base, WPa, host_vt = (
                    lvl["H"], lvl["W"], lvl["R"], lvl["Rw"], lvl["base"], lvl["WPa"], lvl["host_vt"]
                )
                HP = H + 2
                nblk = H // R
                N = R * W  # direct psum free size
                Wh = W // 2
                cls_mode, box_mode = WMODE[li]
                imgs = [0, 1]

                def border_memset(buf):
                    nc.gpsimd.memset(buf[:, :, 0, :], 0.0)
                    nc.gpsimd.memset(buf[:, :, H + 1, :], 0.0)
                    nc.gpsimd.memset(buf[:, :, 1 : H + 1, 0], 0.0)
                    nc.gpsimd.memset(buf[:, :, 1 : H + 1, W + 1], 0.0)

                bufs_, vtx, sbc = {}, {}, {}
                names = ("a1", "b1", "a2", "b2") + (() if host_vt else ("xb",))
                for slot, g in enumerate(imgs):
                    for nmi in names:
                        bufs_[nmi, g] = acts.tile(
                            [128, 2, HP, WPa], F8, name=f"{nmi}{slot}", tag=f"{nmi}{slot}"
                        )
                        border_memset(bufs_[nmi, g])
                    sbc[g] = stage.tile([NCLS, H * W], F32, name=f"sbc{slot}", tag=f"sbc{slot}")
                    if host_vt:
                        vtx[g] = acts.tile(
                            [128, 2, 4, HP, Wh], F8, name=f"vtx{slot}", tag=f"vtx{slot}"
                        )
                        hh = HP // 2
                        for t in range(2):
                            nc.sync.dma_start(
                                out=vtx[g][:, t, :, 0:hh, :],
                                in_=vt_dram[li][g, ts(t, 128), :, 0:hh],
                            )
                            nc.sync.dma_start(
                                out=vtx[g][:, t, :, hh:HP, :],
                                in_=vt_dram[li][g, ts(t, 128), :, hh:HP],
                            )
                    else:
                        hh = HP // 2
                        for t in range(2):
                            nc.sync.dma_start(
                                out=bufs_["xb", g][:, t, 0:hh, :],
                                in_=x_dram[li][g, ts(t, 128), 0:hh],
                            )
                            nc.sync.dma_start(
                                out=bufs_["xb", g][:, t, hh:HP, :],
                                in_=x_dram[li][g, ts(t, 128), hh:HP],
                            )

                # ---- layer helpers ----
                def load_w(dram, lay, wino):
                    if wino:
                        wt = wp.tile([128, 2, 2 * 5 * 3 * 128], F8, name="wtw", tag="wtw", bufs=2)
                    else:
                        wt = wp.tile([128, 2, 2 * 9 * 128], F8, name="wtd", tag="wtd", bufs=2)
                    nc.scalar.dma_start(out=wt[:], in_=dram[lay])
                    return wt

                def make_vt(src, y0, nrows, n_gp):
                    """4 F(2,3) input-transform values for rows y0..y0+nrows-1.
                    The last n_gp ops run on GPSIMD, the rest on the DVE."""
                    vt = [
                        stage.tile([128, 2, nrows, Wh], F8, name=f"vt{i}", tag=f"vt{i}", bufs=2)
                        for i in range(4)
                    ]
                    rows = src[:, :, y0 : y0 + nrows, :]
                    Ej = rows[:, :, :, 0 : W : 2]
                    Ej1 = rows[:, :, :, 2 : W + 2 : 2]
                    Oj = rows[:, :, :, 1 : W + 1 : 2]
                    Oj1 = rows[:, :, :, 3 : W + 3 : 2]
                    args = [
                        (vt[0], Ej, Ej1, sub),
                        (vt[1], Oj, Ej1, add),
                        (vt[2], Ej1, Oj, sub),
                        (vt[3], Oj, Oj1, sub),
                    ]
                    for k, (o, a, b, op) in enumerate(args):
                        eng = nc.gpsimd if k >= 4 - n_gp else nc.vector
                        eng.tensor_tensor(o[:], a, b, op)
                    return vt

                def relus(dst, tower_idx, lay, y0, co, e0, e1):
                    nc.scalar.activation(
                        dst[:, co, 1 + y0 : 1 + y0 + Rw, 1 : W + 1 : 2],
                        e0[:],
                        mybir.ActivationFunctionType.Relu,
                        bias=bst[:, tower_idx, lay, co, :],
                        scale=INV_WSCALE,
                    )
                    nc.scalar.activation(
                        dst[:, co, 1 + y0 : 1 + y0 + Rw, 2 : W + 2 : 2],
                        e1[:],
                        mybir.ActivationFunctionType.Relu,
                        bias=bst[:, tower_idx, lay, co, :],
                        scale=INV_WSCALE,
                    )

                def wino_mms_epilogue(wt, vt_ap, dst, tower_idx, lay, y0):
                    """12 DR matmuls + output transform for one (block, both co).
                    vt_ap(i, dy) -> [128, 2, Rw, Wh] rhs slice."""
                    for co in range(2):
                        ps = [
                            pp.tile([128, Rw, Wh], F32, name=f"ws{i}", tag=f"ws{i}", bufs=2)
                            for i in range(4)
                        ]
                        for i in range(4):
                            for dy in range(3):
                                nc.tensor.matmul(
                                    ps[i][:],
                                    wt[:, :, ts((co * 5 + i) * 3 + dy, 128)],
                                    vt_ap(i, dy),
                                    start=(dy == 0),
                                    stop=(dy == 2),
                                    perf_mode=DRM,
                                )
                        c2 = stage.tile([128, Rw, Wh], F32, name="c2", tag="c2", bufs=2)
                        t0 = stage.tile([128, Rw, Wh], F32, name="t0", tag="t0", bufs=2)
                        e0 = stage.tile([128, Rw, Wh], F32, name="e0", tag="e0", bufs=2)
                        e1x = stage.tile([128, Rw, Wh], F32, name="e1x", tag="e1x", bufs=2)
                        e1 = stage.tile([128, Rw, Wh], F32, name="e1", tag="e1", bufs=2)
                        # e0 = m0+m1+m2; e1 = m1-m2-m3 = t0 - (2*m2 + m3)
                        nc.scalar.activation(c2[:], ps[2][:], mybir.ActivationFunctionType.Copy)
                        nc.vector.tensor_tensor(t0[:], ps[1][:], c2[:], add)
                        nc.vector.tensor_tensor(e0[:], ps[0][:], t0[:], add)
                        nc.vector.scalar_tensor_tensor(e1x[:], c2[:], 2.0, ps[3][:], mult, add)
                        nc.gpsimd.tensor_tensor(e1[:], t0[:], e1x[:], sub)
                        relus(dst, tower_idx, lay, y0, co, e0, e1)

                def wino15_mms_epilogue(wt, vt_ap, dst, tower_idx, lay, y0):
                    """15 DR matmuls accumulating a=m1-m3, b=m2, c=m0+m1 in PSUM;
                    output transform is just e1 = a - b, e0 = c + b (2 DVE ops)."""
                    for co in range(2):
                        # (psum tag, [(weight i-section, vt index), ...])
                        groups = [
                            ("ws0", [(1, 1), (4, 3)]),  # a = m1 - m3
                            ("ws1", [(2, 2)]),          # b = m2
                            ("ws2", [(0, 0), (1, 1)]),  # c = m0 + m1
                        ]
                        pt = {}
                        for tag, parts in groups:
                            p = pp.tile([128, Rw, Wh], F32, name=tag, tag=tag, bufs=2)
                            pt[tag] = p
                            n = len(parts) * 3
                            k = 0
                            for iw, iv in parts:
                                for dy in range(3):
                                    nc.tensor.matmul(
                                        p[:],
                                        wt[:, :, ts((co * 5 + iw) * 3 + dy, 128)],
                                        vt_ap(iv, dy),
                                        start=(k == 0),
                                        stop=(k == n - 1),
                                        perf_mode=DRM,
                                    )
                                    k += 1
                        cb = stage.tile([128, Rw, Wh], F32, name="c2", tag="c2", bufs=2)
                        e0 = stage.tile([128, Rw, Wh], F32, name="e0", tag="e0", bufs=2)
                        e1 = stage.tile([128, Rw, Wh], F32, name="e1", tag="e1", bufs=2)
                        nc.scalar.activation(cb[:], pt["ws1"][:], mybir.ActivationFunctionType.Copy)
                        nc.vector.tensor_tensor(e1[:], pt["ws0"][:], cb[:], sub)
                        nc.vector.tensor_tensor(e0[:], pt["ws2"][:], cb[:], add)
                        relus(dst, tower_idx, lay, y0, co, e0, e1)

                def wino_layer(wt, src, dst, tower_idx, lay, g, mm15, shared_vt=None):
                    made = []
                    for blk in range(H // Rw):
                        y0 = blk * Rw
                        if src is None:  # layer 0: host-precomputed transforms
                            vt_ap = (
                                lambda i, dy, y0=y0: vtx[g][:, :, i, y0 + dy : y0 + dy + Rw, :]
                            )
                        elif shared_vt is not None:
                            vt = shared_vt[blk]
                            vt_ap = lambda i, dy, vt=vt: vt[i][:, :, dy : dy + Rw, :]
                        else:
                            vt = make_vt(src, y0, Rw + 2, n_gp=1 if mm15 else 2)
                            made.append(vt)
                            vt_ap = lambda i, dy, vt=vt: vt[i][:, :, dy : dy + Rw, :]
                        if mm15:
                            wino15_mms_epilogue(wt, vt_ap, dst, tower_idx, lay, y0)
                        else:
                            wino_mms_epilogue(wt, vt_ap, dst, tower_idx, lay, y0)
                    return made

                def direct_layer(wt, src, dst, tower_idx, lay, g):
                    for blk in range(nblk):
                        y0 = blk * R
                        for co in range(2):
                            psd = pp.tile([128, N], F32, name="psd", tag=f"ws{co * 2 + (blk % 2)}", bufs=2)
                            for t, (dy, dx) in enumerate(TAPS):
                                nc.tensor.matmul(
                                    psd[:],
                                    wt[:, :, ts(co * 9 + t, 128)],
                                    src[:, :, y0 + dy : y0 + dy + R, dx : dx + W],
                                    start=(t == 0),
                                    stop=(t == 8),
                                    perf_mode=DRM,
                                )
                            nc.scalar.activation(
                                dst[:, co, 1 + y0 : 1 + y0 + R, 1 : 1 + W],
                                psd[:].rearrange("p (r w) -> p r w", w=W),
                                mybir.ActivationFunctionType.Relu,
                                bias=bst[:, tower_idx, lay, co, :],
                                scale=INV_WSCALE,
                            )

                # ---- stem towers, interleaved per layer ----
                if host_vt:
                    chain = [None, "a1", "b1", "a1", "b1"]
                    chain_b = [None, "a2", "b2", "a2", "b2"]
                else:
                    chain = ["xb", "a1", "b1", "a1", "b1"]
                    chain_b = ["xb", "a2", "b2", "a2", "b2"]
                cls_final, box_final = chain[4], chain_b[4]

                for lay in range(SDEPTH):
                    cm, bm = cls_mode[lay], box_mode[lay]
                    cw, bw = cm in "wf", bm in "wf"
                    wtc = load_w(w_cls_w if cw else w_cls_d, lay, cw)
                    wtb = load_w(w_box_w if bw else w_box_d, lay, bw)
                    for g in imgs:
                        csrc = None if chain[lay] is None else bufs_[chain[lay], g]
                        cdst = bufs_[chain[lay + 1], g]
                        bsrc = None if chain_b[lay] is None else bufs_[chain_b[lay], g]
                        bdst = bufs_[chain_b[lay + 1], g]
                        if cw:
                            made = wino_layer(wtc, csrc, cdst, 0, lay, g, cm == "f")
                        else:
                            direct_layer(wtc, csrc, cdst, 0, lay, g)
                            made = None
                        if bw:
                            # share vt only when both towers read the same src
                            sh = made if (chain[lay] == chain_b[lay] and cw and made) else None
                            wino_layer(wtb, bsrc, bdst, 1, lay, g, bm == "f", shared_vt=sh)
                        else:
                            direct_layer(wtb, bsrc, bdst, 1, lay, g)

                # ---- cls pred: cls_final -> sbc (bias, no relu) ----
                for g in imgs:
                    for blk in range(nblk):
                        y0 = blk * R
                        psc = pp.tile([NCLS, N], F32, name="psc", tag="ws1", bufs=2)
                        for t, (dy, dx) in enumerate(TAPS):
                            nc.tensor.matmul(
                                psc[:],
                                wpc[:, :, ts(t, NCLS)],
                                bufs_[cls_final, g][:, :, y0 + dy : y0 + dy + R, dx : dx + W],
                                start=(t == 0),
                                stop=(t == 8),
                                perf_mode=DRM,
                            )
                        nc.scalar.activation(
                            sbc[g][:, y0 * W : y0 * W + N],
                            psc[:],
                            mybir.ActivationFunctionType.Identity,
                            bias=bp_cls[:],
                            scale=INV_WSCALE,
                        )

                # ---- box+ctr pred from box_final; assemble + write output ----
                for g in imgs:
                    for blk in range(nblk):
                        y0 = blk * R
                        psb = pp.tile([NBC, N], F32, name="psb", tag="ws2", bufs=2)
                        for t, (dy, dx) in enumerate(TAPS):
                            nc.tensor.matmul(
                                psb[:],
                                wpb[:, :, ts(t, NBC)],
                                bufs_[box_final, g][:, :, y0 + dy : y0 + dy + R, dx : dx + W],
                                start=(t == 0),
                                stop=(t == 8),
                                perf_mode=DRM,
                            )
                        sbb = stage.tile([NBC, N], F32, name="sbb", tag="sbb", bufs=2)
                        nc.scalar.activation(
                            sbb[:],
                            psb[:],
                            mybir.ActivationFunctionType.Identity,
                            bias=bp_bc[:],
                            scale=INV_WSCALE,
                        )
                        for c0 in range(0, N, 128):
                            s0 = y0 * W + c0
                            pst = pp.tile([128, 85], F32, name="pst", tag="ws3", bufs=2)
                            nc.tensor.transpose(
                                pst[:, 0:NCLS],
                                sbc[g][:, s0 : s0 + 128],
                                ident[0:NCLS, 0:NCLS],
                            )
                            nc.tensor.transpose(
                                pst[:, NCLS:85],
                                sbb[0:5, c0 : c0 + 128],
                                ident[0:5, 0:5],
                            )
                            osb = stage.tile([128, 85], F32, name="osb", tag="osb", bufs=4)
                            nc.scalar.activation(
                                osb[:], pst[:], mybir.ActivationFunctionType.Copy
                            )
                            nc.sync.dma_start(
                                out=out_dram[g, base + s0 : base + s0 + 128, :],
                                in_=osb[:],
                            )
    return nc


def q8(x):
    return np.clip(x, -240.0, 240.0).astype(NP8)


def prep_weights(inputs):
    """Host-side reshape + fp8 quantization of conv weights into DR lhsT layouts."""
    G = np.array(
        [[1, 0, 0], [0.5, 0.5, 0.5], [0.5, -0.5, 0.5], [0, 0, 1]], np.float64
    )

    def stem_direct(w):  # (S, O, I, 3, 3) -> (S, ci 128, cit 2, cot*tap*co)
        S = w.shape[0]
        t = w.reshape(S, 2, 128, 2, 128, 3, 3)  # (s, cot, co, cit, ci, ky, kx)
        t = t.transpose(0, 4, 3, 1, 5, 6, 2)  # (s, ci, cit, cot, ky, kx, co)
        return np.ascontiguousarray(t.reshape(S, 128, 2, 2 * 9 * 128))

    def stem_wino(w):  # (S, O, I, 3, 3) -> (S, ci, cit, cot*i5*dy3*co)
        S = w.shape[0]
        U = np.einsum("xk,soidk->soixd", G, w.astype(np.float64))  # (S,O,I,4,3)
        U = np.concatenate([U, -U[:, :, :, 3:4, :]], axis=3)  # append -U3 -> (S,O,I,5,3)
        t = U.reshape(S, 2, 128, 2, 128, 5, 3)  # (s, cot, co, cit, ci, x, dy)
        t = t.transpose(0, 4, 3, 1, 5, 6, 2)  # (s, ci, cit, cot, x, dy, co)
        return np.ascontiguousarray(t.reshape(S, 128, 2, 2 * 5 * 3 * 128))

    def pred(w, opad):  # (O, 256, 3, 3) -> (ci 128, cit 2, tap*opad)
        O = w.shape[0]
        wp_ = np.zeros((opad, C, 3, 3), np.float64)
        wp_[:O] = w
        t = wp_.reshape(opad, 2, 128, 3, 3)  # (o, cit, ci, ky, kx)
        t = t.transpose(2, 1, 3, 4, 0)  # (ci, cit, ky, kx, o)
        return np.ascontiguousarray(t.reshape(128, 2, 9 * opad))

    wm = {}
    wm["w_cls_d"] = q8(stem_direct(inputs["stem_cls_w"]) * WSCALE)
    wm["w_box_d"] = q8(stem_direct(inputs["stem_box_w"]) * WSCALE)
    wm["w_cls_w"] = q8(stem_wino(inputs["stem_cls_w"]) * WSCALE)
    wm["w_box_w"] = q8(stem_wino(inputs["stem_box_w"]) * WSCALE)
    wm["w_pcls"] = q8(pred(inputs["pred_cls_w"], NCLS) * WSCALE)
    wm["w_pbc"] = q8(
        pred(
            np.concatenate([inputs["pred_box_w"], inputs["pred_ctr_w"]], axis=0), NBC
        )
        * WSCALE
    )
    bs = np.stack([inputs["stem_cls_b"], inputs["stem_box_b"]], axis=0)  # (2, S, 256)
    bs = bs.reshape(2, SDEPTH, 2, 128).transpose(3, 0, 1, 2)  # (128, 2, S, 2)
    wm["b_stem"] = np.ascontiguousarray(bs.reshape(128, 2 * SDEPTH * 2)).astype(np.float32)
    wm["b_pcls"] = inputs["pred_cls_b"].reshape(NCLS, 1).astype(np.float32)
    bbc = np.zeros((NBC, 1), np.float32)
    bbc[0:4, 0] = inputs["pred_box_b"]
    bbc[4, 0] = inputs["pred_ctr_b"][0]
    wm["b_pbc"] = bbc
    return wm


def host_vt(fq):
    """F(2,3) input transform of fp8-quantized padded input.
    fq: (B, C, HP, WP) float32 holding fp8 values; returns (B, C, 4, HP, W/2) fp8."""
    B_, C_, HP, WP = fq.shape
    W = WP - 2
    E = fq[:, :, :, 0 : W + 1 : 2]
    O = fq[:, :, :, 1 : W + 2 : 2]
    v = np.stack(
        [
            E[:, :, :, :-1] - E[:, :, :, 1:],
            O[:, :, :, :-1] + E[:, :, :, 1:],
            E[:, :, :, 1:] - O[:, :, :, :-1],
            O[:, :, :, :-1] - O[:, :, :, 1:],
        ],
        axis=2,
    )
    return q8(v)


_NC_CACHE = None


def _get_nc():
    global _NC_CACHE
    if _NC_CACHE is None:
        _NC_CACHE = build_nc()
    return _NC_CACHE


def run(inputs, **spmd_kwargs):
    inputs = {k: np.asarray(v) for k, v in inputs.items()}
    nc = _get_nc()
    if not nc.is_finalized():
        nc.finalize()
    wm = prep_weights(inputs)
    in_maps = []
    for core in range(N_CORES):
        m = dict(wm)
        sl = slice(core * BS, (core + 1) * BS)
        for li, lvl in enumerate(LEVELS):
            f = inputs[lvl["feat"]][sl]
            H, W = f.shape[2], f.shape[3]
            if lvl["host_vt"]:
                fp = np.zeros((f.shape[0], f.shape[1], H + 2, W + 2), np.float32)
                fp[:, :, 1:-1, 1:-1] = q8(f).astype(np.float32)
                m[f"vtx_l{li}"] = host_vt(fp)
            else:
                fp = np.zeros((f.shape[0], f.shape[1], H + 2, lvl["WPa"]), NP8)
                fp[:, :, 1:-1, 1 : 1 + W] = q8(f)
                m[f"x_l{li}"] = fp
        in_maps.append(m)
    res = run_bass_kernel_spmd(nc, in_maps, list(range(N_CORES)), **spmd_kwargs)
    out = np.concatenate([res.results[i]["out"] for i in range(N_CORES)], axis=0)
    return out, res


def kernel(**inputs):
    return run(inputs)[0]


# revision 19
# speedup vs baseline: 1.0811x; 1.0811x over previous
"""FCOS detection head on 8 Trainium2 NeuronCores (Bass/Tile), fp8 DoubleRow.

Data parallel: batch 16 -> 2 images per core. Weights replicated.

Per-core compute layout:
  - channels on SBUF partitions (256 ch -> DoubleRow-fused pair of 128-ch
    k-tiles), spatial flattened on the free dim, activations stored fp8(e4m3)
    zero-padded (H+2)x(W+2) in a 16B-aligned pitch so a 3x3 conv is 9
    DoubleRow matmuls (or 12 for the 1D F(2,3) Winograd form) per PSUM tile.
  - weights are scaled by 512 on the host before fp8 quantization; the
    epilogue activation applies 1/512.
  - stem layers run either 1D Winograd F(2,3) along W (input transform split
    DVE/GPSIMD, output transform DVE+GPSIMD+ACT) or direct 3x3 (no DVE work);
    the mix is chosen per tower/layer to balance PE vs DVE load.
  - layer-0 input transforms are precomputed on the HOST (they only depend on
    the network input) and DMA'd in, so layer 0 needs no on-chip transforms.
  - head outputs (85 = 80 cls + 4 box + 1 ctr channels) are assembled
    channels-on-partitions, then PE-transposed per 128-position chunk into
    (positions, 85) and DMA'd to HBM.
"""

import numpy as np
import ml_dtypes

import concourse.bacc as bacc
import concourse.bass as bass
import concourse.mybir as mybir
import concourse.tile as tile
from concourse.bass import ts
from concourse.bass_utils import run_bass_kernel_spmd
from concourse.masks import make_identity

F32 = mybir.dt.float32
F8 = mybir.dt.float8e4
NP8 = ml_dtypes.float8_e4m3
DRM = mybir.MatmulPerfMode.DoubleRow
WSCALE = 512.0
INV_WSCALE = float(1.0 / WSCALE)

N_CORES = 8
B_FULL = 16
BS = B_FULL // N_CORES  # images per core
C = 256
NCLS = 80
NBC = 16  # box+ctr head padded to 16 output channels (5 real)
SDEPTH = 4
TAPS = [(ky, kx) for ky in range(3) for kx in range(3)]

# Winograd/direct mode per (level, tower, layer). 'w' = F(2,3) wino, 'd' = direct.
# levels are emitted p5 first: its direct convs are pure-PE work that runs
# while the (larger) p3/p4 host-vt DMAs stream in, and it warms the PE clock.
# 'w' = 12-matmul F(2,3) wino; 'f' = 15-matmul wino variant that accumulates
# m1+(-m3) and m0+m1 in PSUM so the output transform is only 2 DVE ops;
# 'd' = direct 3x3.
WMODE = [
    ("dddd", "dddd"),  # p5
    ("wwww", "wdfd"),  # p3 (cls, box)
    ("wwww", "wdfd"),  # p4
]

LEVELS = [
    dict(H=16, W=16, R=16, Rw=None, base=5120, WPa=24, host_vt=False, feat="feat_p5"),
    dict(H=64, W=64, R=8, Rw=16, base=0, WPa=72, host_vt=True, feat="feat_p3"),
    dict(H=32, W=32, R=16, Rw=32, base=4096, WPa=40, host_vt=True, feat="feat_p4"),
]
HW_TOTAL = 64 * 64 + 32 * 32 + 16 * 16  # 5376


def build_nc():
    nc = bacc.Bacc()

    x_dram, vt_dram = {}, {}
    for i, lvl in enumerate(LEVELS):
        H, WPa = lvl["H"], lvl["WPa"]
        if lvl["host_vt"]:
            # host-precomputed F(2,3) input transform of x: (B, C, 4i, H+2, W/2)
            vt_dram[i] = nc.declare_dram_parameter(
                f"vtx_l{i}", [BS, C, 4, H + 2, (H and lvl["W"]) // 2], F8, isOutput=False
            )
        else:
            x_dram[i] = nc.declare_dram_parameter(
                f"x_l{i}", [BS, C, H + 2, WPa], F8, isOutput=False
            )
    # direct stem weights: (S, ci 128, ci_t 2, co_t*tap*co)
    w_cls_d = nc.declare_dram_parameter("w_cls_d", [SDEPTH, 128, 2, 2 * 9 * 128], F8, isOutput=False)
    w_box_d = nc.declare_dram_parameter("w_box_d", [SDEPTH, 128, 2, 2 * 9 * 128], F8, isOutput=False)
    # Winograd F(2,3)-transformed stem weights: (S, ci, ci_t, co_t*i5*dy3*co)
    # i-sections 0..3 are U0..U3; section 4 is -U3 (for PSUM-side accumulation)
    w_cls_w = nc.declare_dram_parameter("w_cls_w", [SDEPTH, 128, 2, 2 * 5 * 3 * 128], F8, isOutput=False)
    w_box_w = nc.declare_dram_parameter("w_box_w", [SDEPTH, 128, 2, 2 * 5 * 3 * 128], F8, isOutput=False)
    w_pcls = nc.declare_dram_parameter("w_pcls", [128, 2, 9 * NCLS], F8, isOutput=False)
    w_pbc = nc.declare_dram_parameter("w_pbc", [128, 2, 9 * NBC], F8, isOutput=False)
    b_stem = nc.declare_dram_parameter("b_stem", [128, 2 * SDEPTH * 2], F32, isOutput=False)
    b_pcls = nc.declare_dram_parameter("b_pcls", [NCLS, 1], F32, isOutput=False)
    b_pbc = nc.declare_dram_parameter("b_pbc", [NBC, 1], F32, isOutput=False)
    out_dram = nc.declare_dram_parameter("out", [BS, HW_TOTAL, 85], F32, isOutput=True)

    add = mybir.AluOpType.add
    sub = mybir.AluOpType.subtract
    mult = mybir.AluOpType.mult

    with tile.TileContext(nc) as tc:
        with (
            tc.tile_pool(name="const", bufs=1) as const,
            tc.tile_pool(name="wp", bufs=3) as wp,
            tc.tile_pool(name="acts", bufs=1) as acts,
            tc.tile_pool(name="stage", bufs=1) as stage,
            tc.tile_pool(name="pp", bufs=1, space="PSUM") as pp,
        ):
            ident = const.tile([128, 128], F32, name="ident")
            make_identity(nc, ident[:])
            bst = const.tile([128, 2, SDEPTH, 2, 1], F32, name="bst")
            nc.sync.dma_start(out=bst[:, :, :, :, 0], in_=b_stem[:].rearrange("p (t l c) -> p t l c", t=2, l=SDEPTH, c=2))
            bp_cls = const.tile([NCLS, 1], F32, name="bp_cls")
            nc.sync.dma_start(out=bp_cls[:], in_=b_pcls[:])
            bp_bc = const.tile([NBC, 1], F32, name="bp_bc")
            nc.sync.dma_start(out=bp_bc[:], in_=b_pbc[:])
            wpc = const.tile([128, 2, 9 * NCLS], F8, name="wpc")
            nc.sync.dma_start(out=wpc[:], in_=w_pcls[:])
            wpb = const.tile([128, 2, 9 * NBC], F8, name="wpb")
            nc.sync.dma_start(out=wpb[:], in_=w_pbc[:])

            for li, lvl in enumerate(LEVELS):
                H, W, R, Rw, base, WPa, host_vt = (
                    lvl["H"], lvl["W"], lvl["R"], lvl["Rw"], lvl["base"], lvl["WPa"], lvl["host_vt"]
                )
                HP = H + 2
                nblk = H // R
                N = R * W  # direct psum free size
                Wh = W // 2
                cls_mode, box_mode = WMODE[li]
                imgs = [0, 1]

                def border_memset(buf):
                    nc.gpsimd.memset(buf[:, :, 0, :], 0.0)
                    nc.gpsimd.memset(buf[:, :, H + 1, :], 0.0)
                    nc.gpsimd.memset(buf[:, :, 1 : H + 1, 0], 0.0)
                    nc.gpsimd.memset(buf[:, :, 1 : H + 1, W + 1], 0.0)

                bufs_, vtx, sbc = {}, {}, {}
                names = ("a1", "b1", "a2", "b2") + (() if host_vt else ("xb",))
                for slot, g in enumerate(imgs):
                    for nmi in names:
                        bufs_[nmi, g] = acts.tile(
                            [128, 2, HP, WPa], F8, name=f"{nmi}{slot}", tag=f"{nmi}{slot}"
                        )
                        border_memset(bufs_[nmi, g])
                    sbc[g] = stage.tile([NCLS, H * W], F32, name=f"sbc{slot}", tag=f"sbc{slot}")
                    if host_vt:
                        vtx[g] = acts.tile(
                            [128, 2, 4, HP, Wh], F8, name=f"vtx{slot}", tag=f"vtx{slot}"
                        )
                        hh = HP // 2
                        for t in range(2):
                            nc.sync.dma_start(
                                out=vtx[g][:, t, :, 0:hh, :],
                                in_=vt_dram[li][g, ts(t, 128), :, 0:hh],
                            )
                            nc.sync.dma_start(
                                out=vtx[g][:, t, :, hh:HP, :],
                                in_=vt_dram[li][g, ts(t, 128), :, hh:HP],
                            )
                    else:
                        hh = HP // 2
                        for t in range(2):
                            nc.sync.dma_start(
                                out=bufs_["xb", g][:, t, 0:hh, :],
                                in_=x_dram[li][g, ts(t, 128), 0:hh],
                            )
                            nc.sync.dma_start(
                                out=bufs_["xb", g][:, t, hh:HP, :],
                                in_=x_dram[li][g, ts(t, 128), hh:HP],
                            )

                # ---- layer helpers ----
                def load_w(dram, lay, wino):
                    if wino:
                        wt = wp.tile([128, 2, 2 * 5 * 3 * 128], F8, name="wtw", tag="wtw", bufs=2)
                    else:
                        wt = wp.tile([128, 2, 2 * 9 * 128], F8, name="wtd", tag="wtd", bufs=2)
                    nc.scalar.dma_start(out=wt[:], in_=dram[lay])
                    return wt

                def make_vt(src, y0, nrows, n_gp):
                    """4 F(2,3) input-transform values for rows y0..y0+nrows-1.
                    The last n_gp ops run on GPSIMD, the rest on the DVE."""
                    vt = [
                        stage.tile([128, 2, nrows, Wh], F8, name=f"vt{i}", tag=f"vt{i}", bufs=2)
                        for i in range(4)
                    ]
                    rows = src[:, :, y0 : y0 + nrows, :]
                    Ej = rows[:, :, :, 0 : W : 2]
                    Ej1 = rows[:, :, :, 2 : W + 2 : 2]
                    Oj = rows[:, :, :, 1 : W + 1 : 2]
                    Oj1 = rows[:, :, :, 3 : W + 3 : 2]
                    args = [
                        (vt[0], Ej, Ej1, sub),
                        (vt[1], Oj, Ej1, add),
                        (vt[2], Ej1, Oj, sub),
                        (vt[3], Oj, Oj1, sub),
                    ]
                    for k, (o, a, b, op) in enumerate(args):
                        eng = nc.gpsimd if k >= 4 - n_gp else nc.vector
                        eng.tensor_tensor(o[:], a, b, op)
                    return vt

                def relus(dst, tower_idx, lay, y0, co, e0, e1):
                    nc.scalar.activation(
                        dst[:, co, 1 + y0 : 1 + y0 + Rw, 1 : W + 1 : 2],
                        e0[:],
                        mybir.ActivationFunctionType.Relu,
                        bias=bst[:, tower_idx, lay, co, :],
                        scale=INV_WSCALE,
                    )
                    nc.scalar.activation(
                        dst[:, co, 1 + y0 : 1 + y0 + Rw, 2 : W + 2 : 2],
                        e1[:],
                        mybir.ActivationFunctionType.Relu,
                        bias=bst[:, tower_idx, lay, co, :],
                        scale=INV_WSCALE,
                    )

                def wino_mms_epilogue(wt, vt_ap, dst, tower_idx, lay, y0):
                    """12 DR matmuls + output transform for one (block, both co).
                    vt_ap(i, dy) -> [128, 2, Rw, Wh] rhs slice."""
                    for co in range(2):
                        ps = [
                            pp.tile([128, Rw, Wh], F32, name=f"ws{i}", tag=f"ws{i}", bufs=2)
                            for i in range(4)
                        ]
                        for i in range(4):
                            for dy in range(3):
                                nc.tensor.matmul(
                                    ps[i][:],
                                    wt[:, :, ts((co * 5 + i) * 3 + dy, 128)],
                                    vt_ap(i, dy),
                                    start=(dy == 0),
                                    stop=(dy == 2),
                                    perf_mode=DRM,
                                )
                        c2 = stage.tile([128, Rw, Wh], F32, name="c2", tag="c2", bufs=2)
                        t0 = stage.tile([128, Rw, Wh], F32, name="t0", tag="t0", bufs=2)
                        e0 = stage.tile([128, Rw, Wh], F32, name="e0", tag="e0", bufs=2)
                        e1x = stage.tile([128, Rw, Wh], F32, name="e1x", tag="e1x", bufs=2)
                        e1 = stage.tile([128, Rw, Wh], F32, name="e1", tag="e1", bufs=2)
                        # e0 = m0+m1+m2; e1 = m1-m2-m3 = t0 - (2*m2 + m3)
                        nc.scalar.activation(c2[:], ps[2][:], mybir.ActivationFunctionType.Copy)
                        nc.vector.tensor_tensor(t0[:], ps[1][:], c2[:], add)
                        nc.vector.tensor_tensor(e0[:], ps[0][:], t0[:], add)
                        nc.vector.scalar_tensor_tensor(e1x[:], c2[:], 2.0, ps[3][:], mult, add)
                        nc.gpsimd.tensor_tensor(e1[:], t0[:], e1x[:], sub)
                        relus(dst, tower_idx, lay, y0, co, e0, e1)

                def wino15_mms_epilogue(wt, vt_ap, dst, tower_idx, lay, y0):
                    """15 DR matmuls accumulating a=m1-m3, b=m2, c=m0+m1 in PSUM;
                    output transform is just e1 = a - b, e0 = c + b (2 DVE ops)."""
                    for co in range(2):
                        # (psum tag, [(weight i-section, vt index), ...])
                        groups = [
                            ("ws0", [(1, 1), (4, 3)]),  # a = m1 - m3
                            ("ws1", [(2, 2)]),          # b = m2
                            ("ws2", [(0, 0), (1, 1)]),  # c = m0 + m1
                        ]
                        pt = {}
                        for tag, parts in groups:
                            p = pp.tile([128, Rw, Wh], F32, name=tag, tag=tag, bufs=2)
                            pt[tag] = p
                            n = len(parts) * 3
                            k = 0
                            for iw, iv in parts:
                                for dy in range(3):
                                    nc.tensor.matmul(
                                        p[:],
                                        wt[:, :, ts((co * 5 + iw) * 3 + dy, 128)],
                                        vt_ap(iv, dy),
                                        start=(k == 0),
                                        stop=(k == n - 1),
                                        perf_mode=DRM,
                                    )
                                    k += 1
                        cb = stage.tile([128, Rw, Wh], F32, name="c2", tag="c2", bufs=2)
                        e0 = stage.tile([128, Rw, Wh], F32, name="e0", tag="e0", bufs=2)
                        e1 = stage.tile([128, Rw, Wh], F32, name="e1", tag="e1", bufs=2)
                        nc.scalar.activation(cb[:], pt["ws1"][:], mybir.ActivationFunctionType.Copy)
                        nc.vector.tensor_tensor(e1[:], pt["ws0"][:], cb[:], sub)
                        nc.vector.tensor_tensor(e0[:], pt["ws2"][:], cb[:], add)
                        relus(dst, tower_idx, lay, y0, co, e0, e1)

                def wino_layer(wt, src, dst, tower_idx, lay, g, mm15, shared_vt=None):
                    made = []
                    for blk in range(H // Rw):
                        y0 = blk * Rw
                        if src is None:  # layer 0: host-precomputed transforms
                            vt_ap = (
                                lambda i, dy, y0=y0: vtx[g][:, :, i, y0 + dy : y0 + dy + Rw, :]
                            )
                        elif shared_vt is not None:
                            vt = shared_vt[blk]
                            vt_ap = lambda i, dy, vt=vt: vt[i][:, :, dy : dy + Rw, :]
                        else:
                            vt = make_vt(src, y0, Rw + 2, n_gp=1 if mm15 else 2)
                            made.append(vt)
                            vt_ap = lambda i, dy, vt=vt: vt[i][:, :, dy : dy + Rw, :]
                        if mm15:
                            wino15_mms_epilogue(wt, vt_ap, dst, tower_idx, lay, y0)
                        else:
                            wino_mms_epilogue(wt, vt_ap, dst, tower_idx, lay, y0)
                    return made

                def direct_layer(wt, src, dst, tower_idx, lay, g):
                    for blk in range(nblk):
                        y0 = blk * R
                        for co in range(2):
                            psd = pp.tile([128, N], F32, name="psd", tag=f"ws{co * 2 + (blk % 2)}", bufs=2)
                            for t, (dy, dx) in enumerate(TAPS):
                                nc.tensor.matmul(
                                    psd[:],
                                    wt[:, :, ts(co * 9 + t, 128)],
                                    src[:, :, y0 + dy : y0 + dy + R, dx : dx + W],
                                    start=(t == 0),
                                    stop=(t == 8),
                                    perf_mode=DRM,
                                )
                            nc.scalar.activation(
                                dst[:, co, 1 + y0 : 1 + y0 + R, 1 : 1 + W],
                                psd[:].rearrange("p (r w) -> p r w", w=W),
                                mybir.ActivationFunctionType.Relu,
                                bias=bst[:, tower_idx, lay, co, :],
                                scale=INV_WSCALE,
                            )

                # ---- stem towers, interleaved per layer ----
                if host_vt:
                    chain = [None, "a1", "b1", "a1", "b1"]
                    chain_b = [None, "a2", "b2", "a2", "b2"]
                else:
                    chain = ["xb", "a1", "b1", "a1", "b1"]
                    chain_b = ["xb", "a2", "b2", "a2", "b2"]
                cls_final, box_final = chain[4], chain_b[4]

                for lay in range(SDEPTH):
                    cm, bm = cls_mode[lay], box_mode[lay]
                    cw, bw = cm in "wf", bm in "wf"
                    wtc = load_w(w_cls_w if cw else w_cls_d, lay, cw)
                    wtb = load_w(w_box_w if bw else w_box_d, lay, bw)
                    for g in imgs:
                        csrc = None if chain[lay] is None else bufs_[chain[lay], g]
                        cdst = bufs_[chain[lay + 1], g]
                        bsrc = None if chain_b[lay] is None else bufs_[chain_b[lay], g]
                        bdst = bufs_[chain_b[lay + 1], g]
                        if cw:
                            made = wino_layer(wtc, csrc, cdst, 0, lay, g, cm == "f")
                        else:
                            direct_layer(wtc, csrc, cdst, 0, lay, g)
                            made = None
                        if bw:
                            # share vt only when both towers read the same src
                            sh = made if (chain[lay] == chain_b[lay] and cw and made) else None
                            wino_layer(wtb, bsrc, bdst, 1, lay, g, bm == "f", shared_vt=sh)
                        else:
                            direct_layer(wtb, bsrc, bdst, 1, lay, g)

                # ---- cls pred: cls_final -> sbc (bias, no relu) ----
                for g in imgs:
                    for blk in range(nblk):
                        y0 = blk * R
                        psc = pp.tile([NCLS, N], F32, name="psc", tag="ws1", bufs=2)
                        for t, (dy, dx) in enumerate(TAPS):
                            nc.tensor.matmul(
                                psc[:],
                                wpc[:, :, ts(t, NCLS)],
                                bufs_[cls_final, g][:, :, y0 + dy : y0 + dy + R, dx : dx + W],
                                start=(t == 0),
                                stop=(t == 8),
                                perf_mode=DRM,
                            )
                        nc.scalar.activation(
                            sbc[g][:, y0 * W : y0 * W + N],
                            psc[:],
                            mybir.ActivationFunctionType.Identity,
                            bias=bp_cls[:],
                            scale=INV_WSCALE,
                        )

                # ---- box+ctr pred from box_final; assemble + write output ----
                for g in imgs:
                    for blk in range(nblk):
                        y0 = blk * R
                        psb = pp.tile([NBC, N], F32, name="psb", tag="ws2", bufs=2)
                        for t, (dy, dx) in enumerate(TAPS):
                            nc.tensor.matmul(
                                psb[:],
                                wpb[:, :, ts(t, NBC)],
                                bufs_[box_final, g][:, :, y0 + dy : y0 + dy + R, dx : dx + W],
                                start=(t == 0),
                                stop=(t == 8),
                                perf_mode=DRM,
                            )
                        sbb = stage.tile([NBC, N], F32, name="sbb", tag="sbb", bufs=2)
                        nc.scalar.activation(
                            sbb[:],
                            psb[:],
                            mybir.ActivationFunctionType.Identity,
                            bias=bp_bc[:],
                            scale=INV_WSCALE,
                        )
                        for c0 in range(0, N, 128):
                            s0 = y0 * W + c0
                            pst = pp.tile([128, 85], F32, name="pst", tag="ws3", bufs=2)
                            nc.tensor.transpose(
                                pst[:, 0:NCLS],
                                sbc[g][:, s0 : s0 + 128],
                                ident[0:NCLS, 0:NCLS],
                            )
                            nc.tensor.transpose(
                                pst[:, NCLS:85],
                                sbb[0:5, c0 : c0 + 128],
                                ident[0:5, 0:5],
                            )
                            osb = stage.tile([128, 85], F32, name="osb", tag="osb", bufs=4)
                            nc.scalar.activation(
                                osb[:], pst[:], mybir.ActivationFunctionType.Copy
                            )
                            nc.sync.dma_start(
                                out=out_dram[g, base + s0 : base + s0 + 128, :],
                                in_=osb[:],
                            )
    return nc


def q8(x):
    return np.clip(x, -240.0, 240.0).astype(NP8)


def prep_weights(inputs):
    """Host-side reshape + fp8 quantization of conv weights into DR lhsT layouts."""
    G = np.array(
        [[1, 0, 0], [0.5, 0.5, 0.5], [0.5, -0.5, 0.5], [0, 0, 1]], np.float64
    )

    def stem_direct(w):  # (S, O, I, 3, 3) -> (S, ci 128, cit 2, cot*tap*co)
        S = w.shape[0]
        t = w.reshape(S, 2, 128, 2, 128, 3, 3)  # (s, cot, co, cit, ci, ky, kx)
        t = t.transpose(0, 4, 3, 1, 5, 6, 2)  # (s, ci, cit, cot, ky, kx, co)
        return np.ascontiguousarray(t.reshape(S, 128, 2, 2 * 9 * 128))

    def stem_wino(w):  # (S, O, I, 3, 3) -> (S, ci, cit, cot*i5*dy3*co)
        S = w.shape[0]
        U = np.einsum("xk,soidk->soixd", G, w.astype(np.float64))  # (S,O,I,4,3)
        U = np.concatenate([U, -U[:, :, :, 3:4, :]], axis=3)  # append -U3 -> (S,O,I,5,3)
        t = U.reshape(S, 2, 128, 2, 128, 5, 3)  # (s, cot, co, cit, ci, x, dy)
        t = t.transpose(0, 4, 3, 1, 5, 6, 2)  # (s, ci, cit, cot, x, dy, co)
        return np.ascontiguousarray(t.reshape(S, 128, 2, 2 * 5 * 3 * 128))

    def pred(w, opad):  # (O, 256, 3, 3) -> (ci 128, cit 2, tap*opad)
        O = w.shape[0]
        wp_ = np.zeros((opad, C, 3, 3), np.float64)
        wp_[:O] = w
        t = wp_.reshape(opad, 2, 128, 3, 3)  # (o, cit, ci, ky, kx)
        t = t.transpose(2, 1, 3, 4, 0)  # (ci, cit, ky, kx, o)
        return np.ascontiguousarray(t.reshape(128, 2, 9 * opad))

    wm = {}
    wm["w_cls_d"] = q8(stem_direct(inputs["stem_cls_w"]) * WSCALE)
    wm["w_box_d"] = q8(stem_direct(inputs["stem_box_w"]) * WSCALE)
    wm["w_cls_w"] = q8(stem_wino(inputs["stem_cls_w"]) * WSCALE)
    wm["w_box_w"] = q8(stem_wino(inputs["stem_box_w"]) * WSCALE)
    wm["w_pcls"] = q8(pred(inputs["pred_cls_w"], NCLS) * WSCALE)
    wm["w_pbc"] = q8(
        pred(
            np.concatenate([inputs["pred_box_w"], inputs["pred_ctr_w"]], axis=0), NBC
        )
        * WSCALE
    )
    bs = np.stack([inputs["stem_cls_b"], inputs["stem_box_b"]], axis=0)  # (2, S, 256)
    bs = bs.reshape(2, SDEPTH, 2, 128).transpose(3, 0, 1, 2)  # (128, 2, S, 2)
    wm["b_stem"] = np.ascontiguousarray(bs.reshape(128, 2 * SDEPTH * 2)).astype(np.float32)
    wm["b_pcls"] = inputs["pred_cls_b"].reshape(NCLS, 1).astype(np.float32)
    bbc = np.zeros((NBC, 1), np.float32)
    bbc[0:4, 0] = inputs["pred_box_b"]
    bbc[4, 0] = inputs["pred_ctr_b"][0]
    wm["b_pbc"] = bbc
    return wm


def host_vt(fq):
    """F(2,3) input transform of fp8-quantized padded input.
    fq: (B, C, HP, WP) float32 holding fp8 values; returns (B, C, 4, HP, W/2) fp8."""
    B_, C_, HP, WP = fq.shape
    W = WP - 2
    E = fq[:, :, :, 0 : W + 1 : 2]
    O = fq[:, :, :, 1 : W + 2 : 2]
    v = np.stack(
        [
            E[:, :, :, :-1] - E[:, :, :, 1:],
            O[:, :, :, :-1] + E[:, :, :, 1:],
            E[:, :, :, 1:] - O[:, :, :, :-1],
            O[:, :, :, :-1] - O[:, :, :, 1:],
        ],
        axis=2,
    )
    return q8(v)


_NC_CACHE = None


def _get_nc():
    global _NC_CACHE
    if _NC_CACHE is None:
        _NC_CACHE = build_nc()
    return _NC_CACHE


def run(inputs, **spmd_kwargs):
    inputs = {k: np.asarray(v) for k, v in inputs.items()}
    nc = _get_nc()
    if not nc.is_finalized():
        nc.finalize()
    wm = prep_weights(inputs)
    in_maps = []
    for core in range(N_CORES):
        m = dict(wm)
        sl = slice(core * BS, (core + 1) * BS)
        for li, lvl in enumerate(LEVELS):
            f = inputs[lvl["feat"]][sl]
            H, W = f.shape[2], f.shape[3]
            if lvl["host_vt"]:
                fp = np.zeros((f.shape[0], f.shape[1], H + 2, W + 2), np.float32)
                fp[:, :, 1:-1, 1:-1] = q8(f).astype(np.float32)
                m[f"vtx_l{li}"] = host_vt(fp)
            else:
                fp = np.zeros((f.shape[0], f.shape[1], H + 2, lvl["WPa"]), NP8)
                fp[:, :, 1:-1, 1 : 1 + W] = q8(f)
                m[f"x_l{li}"] = fp
        in_maps.append(m)
    res = run_bass_kernel_spmd(nc, in_maps, list(range(N_CORES)), **spmd_kwargs)
    out = np.concatenate([res.results[i]["out"] for i in range(N_CORES)], axis=0)
    return out, res


def kernel(**inputs):
    return run(inputs)[0]
